# revision 10
# baseline (speedup 1.0000x reference)
"""Trainium2 Bass kernel for nn_CombinedRotaryEmbedding.

v4: the whole op is two cascaded matmuls -- no elementwise stage.

The reference is out = x . M . P(s) per 64-dim head row, where M folds
the 32 Givens rotations + rotation_matrix, and P(s) is the RoPE mixing
(split-even/odd, position-dependent cos/sin).  P(s) = D(s) . Pi with
D(s) block-diagonal 2x2 rotations (angle s*inv_freq[c]) and Pi a fixed
permutation.  Rotations compose: D(a+b) = D(a)D(b).  With s = base(core)
+ 32h + l (l, h in [0,32)):

    out = (x . [M D(base+l)]) . [D(32h) Pi]
           stage 1: 32 weights     stage 2: 32 weights

Each stage is 32 big matmuls (one per l resp. h over 1024 columns), so
the only non-PE work is PSUM->SBUF copies (Act/DVE split).  fp16 I/O
halves HBM traffic (~17 MB/core); total rel err ~4.4e-4 vs the 2e-2
gate.

Sharding: sequence-parallel over 8 cores (1024 positions each).  Host
pre-transposes x to free layout (l, q=(b,hi), h) and inverse-permutes
the output from (h, q, l).
"""

import numpy as np


def _import_bass():
    try:
        import concourse.bass  # noqa: F401
    except ImportError:
        import sys

        sys.path.insert(0, "/opt/trn_rl_repo")


_import_bass()

import concourse.bass as bass  # noqa: E402
import concourse.mybir as mybir  # noqa: E402
from concourse.tile import TileContext  # noqa: E402
from concourse.vector_clock import ScopedClock  # noqa: E402

B, S, NSTATE = 4, 8192, 1024
H, D, NUM_ROT = 16, 64, 32
NCORES = 8
S_SH = S // NCORES  # 1024 positions per core
FREE = B * (H // 2) * S_SH  # 32768 columns per core
CHUNK = 4096

F32 = mybir.dt.float32
F16 = mybir.dt.float16


class _TileContextSplitDrain(TileContext):
    """TileContext whose final drain carries at most one sem wait per
    instruction — the walrus in this container rejects instructions
    with 2+ sync waits ("Too many sync wait commands")."""

    def _drain_and_barrier(self, tick_clock, wait_clock):
        nc = self.nc
        drain_inst = nc.sync.drain()
        wait_clock.add_sem_waits(
            drain_inst.ins, ScopedClock({None: tick_clock.global_clock})
        )
        si = drain_inst.ins.sync_info
        waits = list(si.on_wait or [])
        if len(waits) > 1:
            si.on_wait = [waits[0]]
            for w in waits[1:]:
                n = nc.sync.nop(nofuse=True, hint="drain_wait_split")
                n.ins.sync_info = type(si)(on_update=[], on_wait=[w])
        nc.all_engine_barrier()
        assert self.sems is not None
        popped = nc._tile_sem_poison_stack.pop()
        assert popped is self._sem_poison
        nc.clear_and_free_semaphores(list(self.sems.allocated().values()))
        nc.all_engine_barrier()


def _split_excess_waits(nc, limit=1):
    """Walrus here rejects instructions with >limit sync waits.  Hoist
    excess waits onto same-engine InstNoOps inserted immediately before
    the offending instruction (same engine stream => program order)."""
    n_split = 0
    for fn in nc.m.functions:
        for blk in fn.blocks:
            insts = blk.instructions
            i = 0
            while i < len(insts):
                inst = insts[i]
                si = getattr(inst, "sync_info", None)
                waits = list(si.on_wait) if (si and si.on_wait) else []
                if len(waits) > limit:
                    keep = waits[-limit:]
                    excess = waits[:-limit]
                    si.on_wait = keep
                    for j, w in enumerate(excess):
                        nop = mybir.InstNoOp(
                            name=f"{inst.name}-wsplit{j}",
                            engine=inst.engine,
                            bass_nofuse=True,
                            sync_info=mybir.SyncInfo(on_wait=[w], on_update=[]),
                        )
                        insts.insert(i, nop)
                        i += 1
                        n_split += 1
                i += 1
    return n_split


def compose_rotation(thetas: np.ndarray, rotation_matrix: np.ndarray) -> np.ndarray:
    """Fold the sequential Givens rotations + rotation_matrix into one 64x64."""
    M = np.eye(D, dtype=np.float64)
    th = thetas.astype(np.float64)
    for k in range(NUM_ROT):
        i, j = k % D, (k + 1) % D
        c, s = np.cos(th[k]), np.sin(th[k])
        mi = M[:, i] * c + M[:, j] * s
        mj = -M[:, i] * s + M[:, j] * c
        M[:, i], M[:, j] = mi, mj
    return M @ rotation_matrix.astype(np.float64)


def _dmat(mult: float, invf: np.ndarray) -> np.ndarray:
    """64x64 block-diag over channel pairs (2c,2c+1): [[cos,sin],[-sin,cos]]
    with angle mult*invf[c] (row-vector convention)."""
    Dm = np.zeros((D, D))
    ang = float(mult) * invf.astype(np.float64)
    co, si = np.cos(ang), np.sin(ang)
    for c in range(32):
        Dm[2 * c, 2 * c] = co[c]
        Dm[2 * c, 2 * c + 1] = si[c]
        Dm[2 * c + 1, 2 * c] = -si[c]
        Dm[2 * c + 1, 2 * c + 1] = co[c]
    return Dm


def _pi() -> np.ndarray:
    """RoPE output permutation: channel 2c -> c, 2c+1 -> 32+c."""
    P = np.zeros((D, D))
    for c in range(32):
        P[2 * c, c] = 1.0
        P[2 * c + 1, 32 + c] = 1.0
    return P


def _pack2(W: np.ndarray) -> np.ndarray:
    """[64,64] -> [128,128] block-diag x2 (two head-halves share weights)."""
    Z = np.zeros((128, 128))
    Z[0:64, 0:64] = W
    Z[64:128, 64:128] = W
    return Z


def build_weights(thetas, rotation_matrix, inv_freq):
    """Per-core stage-1 stacks [8,128,4096] and shared stage-2 [128,4096]."""
    M = compose_rotation(thetas, rotation_matrix)
    invf = inv_freq.astype(np.float64)
    wa = np.empty((NCORES, 128, 32 * 128), dtype=np.float16)
    for c in range(NCORES):
        base = S_SH * c
        for l in range(32):
            wa[c, :, l * 128 : (l + 1) * 128] = _pack2(M @ _dmat(base + l, invf))
    Pi = _pi()
    wb = np.empty((128, 32 * 128), dtype=np.float16)
    for h in range(32):
        wb[:, h * 128 : (h + 1) * 128] = _pack2(_dmat(32 * h, invf) @ Pi)
    return wa, wb


def shard_x(x: np.ndarray) -> np.ndarray:
    """[B,S,1024] f32 -> [core, 128 (hp,d), 32768 (l, q=(b,hi), h)] fp16."""
    xr = np.ascontiguousarray(x).reshape(B, NCORES, 32, 32, H // 2, 2, D)
    # dims: b, core, h, l, hi, hp, d  ->  core, hp, d, l, b, hi, h
    xt = xr.transpose(1, 5, 6, 3, 0, 4, 2)
    return np.ascontiguousarray(xt, dtype=np.float16).reshape(NCORES, 128, FREE)


def unshard_out(o: np.ndarray) -> np.ndarray:
    """[core, 128 (hp,c), 32768 (h, q=(b,hi), l)] fp16 -> [B,S,1024] f32."""
    orr = o.astype(np.float32).reshape(NCORES, 2, 64, 32, B, 8, 32)
    # dims: core, hp, c, h, b, hi, l -> b, (core,h,l)=s, (hi,hp,c)=n
    ot = orr.transpose(4, 0, 3, 6, 5, 1, 2)
    return np.ascontiguousarray(ot).reshape(B, S, NSTATE)


_NC_CACHE = {}


def _build_nc():
    if "nc" in _NC_CACHE:
        return _NC_CACHE["nc"]
    nc = bass.Bass(trn_type="TRN2")
    x_d = nc.dram_tensor("x", [128, FREE], F16, kind="ExternalInput")
    wa_d = nc.dram_tensor("wa", [128, 4096], F16, kind="ExternalInput")
    wb_d = nc.dram_tensor("wb", [128, 4096], F16, kind="ExternalInput")
    o_d = nc.dram_tensor("o", [128, FREE], F16, kind="ExternalOutput")

    with _TileContextSplitDrain(nc) as tc:
        with tc.tile_pool(name="const", bufs=1) as cpool, \
             tc.tile_pool(name="xin", bufs=3) as xpool, \
             tc.tile_pool(name="ys", bufs=1) as ypool, \
             tc.tile_pool(name="oout", bufs=2) as opool, \
             tc.tile_pool(name="ps1", bufs=2, space="PSUM") as p1pool, \
             tc.tile_pool(name="ps2", bufs=2, space="PSUM") as p2pool:
            wa = cpool.tile([128, 4096], F16, tag="wa")
            wb = cpool.tile([128, 4096], F16, tag="wb")
            nc.sync.dma_start(out=wa, in_=wa_d.ap())
            nc.sync.dma_start(out=wb, in_=wb_d.ap())

            # ys free layout m*1024 + q*32 + l (m-major) so stage-2 rhs is
            # CONTIGUOUS -- strided matmul rhs measured +70% PE time.  The
            # scatter moves into the stage-1 drain instead (copies tolerate
            # strides; the PE does not).
            ys = ypool.tile([128, FREE], F16, tag="ys")
            ysv = ys.rearrange("p (m q l) -> p m q l", m=32, q=32, l=32)

            # Stage 1: per l-block, u = x . M D(base+l); scatter into ys.
            for ch in range(FREE // CHUNK):
                xt = xpool.tile([128, CHUNK], F16)
                nc.sync.dma_start(
                    out=xt, in_=x_d.ap()[:, ch * CHUNK : (ch + 1) * CHUNK]
                )
                for j in range(CHUNK // 1024):
                    l = ch * (CHUNK // 1024) + j
                    w_l = wa[:, l * 128 : (l + 1) * 128]
                    ps1 = p1pool.tile([128, 1024], F32)
                    nc.tensor.matmul(
                        ps1[:, 0:512], lhsT=w_l,
                        rhs=xt[:, j * 1024 : j * 1024 + 512],
                        start=True, stop=True,
                    )
                    nc.tensor.matmul(
                        ps1[:, 512:1024], lhsT=w_l,
                        rhs=xt[:, j * 1024 + 512 : (j + 1) * 1024],
                        start=True, stop=True,
                    )
                    p1t = ps1.rearrange("p (q m) -> p m q", q=32, m=32)
                    nc.scalar.copy(
                        out=ysv[:, :, 0:16, l], in_=p1t[:, :, 0:16]
                    )
                    nc.vector.tensor_copy(
                        out=ysv[:, :, 16:32, l], in_=p1t[:, :, 16:32]
                    )

            # Stage 2: per h-block, out = ys_h . D(32h) Pi; contiguous rhs
            # and contiguous drain into ot (free layout h*1024 + q*32 + l).
            for hc in range(FREE // CHUNK):
                ot = opool.tile([128, CHUNK], F16)
                for j in range(CHUNK // 1024):
                    h = hc * (CHUNK // 1024) + j
                    w_h = wb[:, h * 128 : (h + 1) * 128]
                    ps2 = p2pool.tile([128, 1024], F32)
                    nc.tensor.matmul(
                        ps2[:, 0:512], lhsT=w_h,
                        rhs=ys[:, h * 1024 : h * 1024 + 512],
                        start=True, stop=True,
                    )
                    nc.tensor.matmul(
                        ps2[:, 512:1024], lhsT=w_h,
                        rhs=ys[:, h * 1024 + 512 : (h + 1) * 1024],
                        start=True, stop=True,
                    )
                    nc.scalar.copy(
                        out=ot[:, j * 1024 : j * 1024 + 512],
                        in_=ps2[:, 0:512],
                    )
                    nc.vector.tensor_copy(
                        out=ot[:, j * 1024 + 512 : (j + 1) * 1024],
                        in_=ps2[:, 512:1024],
                    )
                nc.sync.dma_start(
                    out=o_d.ap()[:, hc * CHUNK : (hc + 1) * CHUNK], in_=ot
                )
    _split_excess_waits(nc)
    _NC_CACHE["nc"] = nc
    return nc


def kernel(x, thetas, rotation_matrix, inv_freq, _trace=False):
    from concourse.bass_utils import run_bass_kernel_spmd

    x = np.asarray(x, dtype=np.float32)
    thetas = np.asarray(thetas, dtype=np.float32)
    rotation_matrix = np.asarray(rotation_matrix, dtype=np.float32)
    inv_freq = np.asarray(inv_freq, dtype=np.float32)

    wa, wb = build_weights(thetas, rotation_matrix, inv_freq)
    xs = shard_x(x)

    nc = _build_nc()
    in_maps = [
        {"x": xs[c], "wa": wa[c], "wb": wb} for c in range(NCORES)
    ]
    res = run_bass_kernel_spmd(
        nc, in_maps, core_ids=list(range(NCORES)), trace=_trace
    )
    o = np.stack([res.results[c]["o"] for c in range(NCORES)])
    out = unshard_out(o)
    if _trace:
        return out, res
    return out


# revision 11
# speedup vs baseline: 1.5647x; 1.5647x over previous
"""Trainium2 Bass kernel for nn_CombinedRotaryEmbedding.

v4: the whole op is two cascaded matmuls -- no elementwise stage.

The reference is out = x . M . P(s) per 64-dim head row, where M folds
the 32 Givens rotations + rotation_matrix, and P(s) is the RoPE mixing
(split-even/odd, position-dependent cos/sin).  P(s) = D(s) . Pi with
D(s) block-diagonal 2x2 rotations (angle s*inv_freq[c]) and Pi a fixed
permutation.  Rotations compose: D(a+b) = D(a)D(b).  With s = base(core)
+ 32h + l (l, h in [0,32)):

    out = (x . [M D(base+l)]) . [D(32h) Pi]
           stage 1: 32 weights     stage 2: 32 weights

Each stage is 32 big matmuls (one per l resp. h over 1024 columns), so
the only non-PE work is PSUM->SBUF copies (Act/DVE split).  fp16 I/O
halves HBM traffic (~17 MB/core); total rel err ~4.4e-4 vs the 2e-2
gate.

Sharding: sequence-parallel over 8 cores (1024 positions each).  Host
pre-transposes x to free layout (l, q=(b,hi), h) and inverse-permutes
the output from (h, q, l).
"""

import numpy as np


def _import_bass():
    try:
        import concourse.bass  # noqa: F401
    except ImportError:
        import sys

        sys.path.insert(0, "/opt/trn_rl_repo")


_import_bass()

import concourse.bass as bass  # noqa: E402
import concourse.mybir as mybir  # noqa: E402
from concourse.tile import TileContext  # noqa: E402
from concourse.vector_clock import ScopedClock  # noqa: E402

B, S, NSTATE = 4, 8192, 1024
H, D, NUM_ROT = 16, 64, 32
NCORES = 8
S_SH = S // NCORES  # 1024 positions per core
FREE = B * (H // 2) * S_SH  # 32768 columns per core
CHUNK = 4096

F32 = mybir.dt.float32
F16 = mybir.dt.float16


class _TileContextSplitDrain(TileContext):
    """TileContext whose final drain carries at most one sem wait per
    instruction — the walrus in this container rejects instructions
    with 2+ sync waits ("Too many sync wait commands")."""

    def _drain_and_barrier(self, tick_clock, wait_clock):
        nc = self.nc
        drain_inst = nc.sync.drain()
        wait_clock.add_sem_waits(
            drain_inst.ins, ScopedClock({None: tick_clock.global_clock})
        )
        si = drain_inst.ins.sync_info
        waits = list(si.on_wait or [])
        if len(waits) > 1:
            si.on_wait = [waits[0]]
            for w in waits[1:]:
                n = nc.sync.nop(nofuse=True, hint="drain_wait_split")
                n.ins.sync_info = type(si)(on_update=[], on_wait=[w])
        nc.all_engine_barrier()
        assert self.sems is not None
        popped = nc._tile_sem_poison_stack.pop()
        assert popped is self._sem_poison
        nc.clear_and_free_semaphores(list(self.sems.allocated().values()))
        nc.all_engine_barrier()


def _split_excess_waits(nc, limit=1):
    """Walrus here rejects instructions with >limit sync waits.  Hoist
    excess waits onto same-engine InstNoOps inserted immediately before
    the offending instruction (same engine stream => program order)."""
    n_split = 0
    for fn in nc.m.functions:
        for blk in fn.blocks:
            insts = blk.instructions
            i = 0
            while i < len(insts):
                inst = insts[i]
                si = getattr(inst, "sync_info", None)
                waits = list(si.on_wait) if (si and si.on_wait) else []
                if len(waits) > limit:
                    keep = waits[-limit:]
                    excess = waits[:-limit]
                    si.on_wait = keep
                    for j, w in enumerate(excess):
                        nop = mybir.InstNoOp(
                            name=f"{inst.name}-wsplit{j}",
                            engine=inst.engine,
                            bass_nofuse=True,
                            sync_info=mybir.SyncInfo(on_wait=[w], on_update=[]),
                        )
                        insts.insert(i, nop)
                        i += 1
                        n_split += 1
                i += 1
    return n_split


def compose_rotation(thetas: np.ndarray, rotation_matrix: np.ndarray) -> np.ndarray:
    """Fold the sequential Givens rotations + rotation_matrix into one 64x64."""
    M = np.eye(D, dtype=np.float64)
    th = thetas.astype(np.float64)
    for k in range(NUM_ROT):
        i, j = k % D, (k + 1) % D
        c, s = np.cos(th[k]), np.sin(th[k])
        mi = M[:, i] * c + M[:, j] * s
        mj = -M[:, i] * s + M[:, j] * c
        M[:, i], M[:, j] = mi, mj
    return M @ rotation_matrix.astype(np.float64)


def _dmat(mult: float, invf: np.ndarray) -> np.ndarray:
    """64x64 block-diag over channel pairs (2c,2c+1): [[cos,sin],[-sin,cos]]
    with angle mult*invf[c] (row-vector convention)."""
    Dm = np.zeros((D, D))
    ang = float(mult) * invf.astype(np.float64)
    co, si = np.cos(ang), np.sin(ang)
    for c in range(32):
        Dm[2 * c, 2 * c] = co[c]
        Dm[2 * c, 2 * c + 1] = si[c]
        Dm[2 * c + 1, 2 * c] = -si[c]
        Dm[2 * c + 1, 2 * c + 1] = co[c]
    return Dm


def _pi() -> np.ndarray:
    """RoPE output permutation: channel 2c -> c, 2c+1 -> 32+c."""
    P = np.zeros((D, D))
    for c in range(32):
        P[2 * c, c] = 1.0
        P[2 * c + 1, 32 + c] = 1.0
    return P


def _pack2(W: np.ndarray) -> np.ndarray:
    """[64,64] -> [128,128] block-diag x2 (two head-halves share weights)."""
    Z = np.zeros((128, 128))
    Z[0:64, 0:64] = W
    Z[64:128, 64:128] = W
    return Z


def build_weights(thetas, rotation_matrix, inv_freq):
    """Per-core stage-1 stacks [8,128,4096] and shared stage-2 [128,4096]."""
    M = compose_rotation(thetas, rotation_matrix)
    invf = inv_freq.astype(np.float64)
    wa = np.empty((NCORES, 128, 32 * 128), dtype=np.float16)
    for c in range(NCORES):
        base = S_SH * c
        for l in range(32):
            wa[c, :, l * 128 : (l + 1) * 128] = _pack2(M @ _dmat(base + l, invf))
    Pi = _pi()
    wb = np.empty((128, 32 * 128), dtype=np.float16)
    for h in range(32):
        wb[:, h * 128 : (h + 1) * 128] = _pack2(_dmat(32 * h, invf) @ Pi)
    return wa, wb


def shard_x(x: np.ndarray) -> np.ndarray:
    """[B,S,1024] f32 -> [core, 128 (hp,d), 32768 (l, h, q=(b,hi))] fp16."""
    xr = np.ascontiguousarray(x).reshape(B, NCORES, 32, 32, H // 2, 2, D)
    # dims: b, core, h, l, hi, hp, d  ->  core, hp, d, l, h, b, hi
    xt = xr.transpose(1, 5, 6, 3, 2, 0, 4)
    return np.ascontiguousarray(xt, dtype=np.float16).reshape(NCORES, 128, FREE)


def unshard_out(o: np.ndarray) -> np.ndarray:
    """[core, 128 (hp,c), 32768 (h, l, q=(b,hi))] fp16 -> [B,S,1024] f32."""
    orr = o.astype(np.float32).reshape(NCORES, 2, 64, 32, 32, B, 8)
    # dims: core, hp, c, h, l, b, hi -> b, (core,h,l)=s, (hi,hp,c)=n
    ot = orr.transpose(5, 0, 3, 4, 6, 1, 2)
    return np.ascontiguousarray(ot).reshape(B, S, NSTATE)


_NC_CACHE = {}


def _build_nc():
    if "nc" in _NC_CACHE:
        return _NC_CACHE["nc"]
    nc = bass.Bass(trn_type="TRN2")
    x_d = nc.dram_tensor("x", [128, FREE], F16, kind="ExternalInput")
    wa_d = nc.dram_tensor("wa", [128, 4096], F16, kind="ExternalInput")
    wb_d = nc.dram_tensor("wb", [128, 4096], F16, kind="ExternalInput")
    o_d = nc.dram_tensor("o", [128, FREE], F16, kind="ExternalOutput")

    with _TileContextSplitDrain(nc) as tc:
        with tc.tile_pool(name="const", bufs=1) as cpool, \
             tc.tile_pool(name="xin", bufs=3) as xpool, \
             tc.tile_pool(name="ys", bufs=1) as ypool, \
             tc.tile_pool(name="oout", bufs=2) as opool, \
             tc.tile_pool(name="ps1", bufs=2, space="PSUM") as p1pool, \
             tc.tile_pool(name="ps2", bufs=2, space="PSUM") as p2pool:
            wa = cpool.tile([128, 4096], F16, tag="wa")
            wb = cpool.tile([128, 4096], F16, tag="wb")
            nc.sync.dma_start(out=wa, in_=wa_d.ap())
            nc.sync.dma_start(out=wb, in_=wb_d.ap())

            # ys free layout m*1024 + q*32 + l (m-major) so stage-2 rhs is
            # CONTIGUOUS -- strided matmul rhs measured +70% PE time.  The
            # scatter moves into the stage-1 drain instead (copies tolerate
            # strides; the PE does not).
            ys = ypool.tile([128, FREE], F16, tag="ys")
            ysv = ys.rearrange("p (m l q) -> p m l q", m=32, l=32, q=32)

            # Stage 1: per l-block, u = x . M D(base+l); scatter into ys.
            for ch in range(FREE // CHUNK):
                xt = xpool.tile([128, CHUNK], F16)
                nc.sync.dma_start(
                    out=xt, in_=x_d.ap()[:, ch * CHUNK : (ch + 1) * CHUNK]
                )
                for j in range(CHUNK // 1024):
                    l = ch * (CHUNK // 1024) + j
                    w_l = wa[:, l * 128 : (l + 1) * 128]
                    ps1 = p1pool.tile([128, 1024], F32)
                    nc.tensor.matmul(
                        ps1[:, 0:512], lhsT=w_l,
                        rhs=xt[:, j * 1024 : j * 1024 + 512],
                        start=True, stop=True,
                    )
                    nc.tensor.matmul(
                        ps1[:, 512:1024], lhsT=w_l,
                        rhs=xt[:, j * 1024 + 512 : (j + 1) * 1024],
                        start=True, stop=True,
                    )
                    p1v = ps1.rearrange("p (m q) -> p m q", m=32, q=32)
                    nc.scalar.copy(
                        out=ysv[:, 0:16, l, :], in_=p1v[:, 0:16, :]
                    )
                    nc.vector.tensor_copy(
                        out=ysv[:, 16:32, l, :], in_=p1v[:, 16:32, :]
                    )

            # Stage 2: per h-block, out = ys_h . D(32h) Pi; contiguous rhs
            # and contiguous drain into ot (free layout h*1024 + q*32 + l).
            for hc in range(FREE // CHUNK):
                ot = opool.tile([128, CHUNK], F16)
                for j in range(CHUNK // 1024):
                    h = hc * (CHUNK // 1024) + j
                    w_h = wb[:, h * 128 : (h + 1) * 128]
                    ps2 = p2pool.tile([128, 1024], F32)
                    nc.tensor.matmul(
                        ps2[:, 0:512], lhsT=w_h,
                        rhs=ys[:, h * 1024 : h * 1024 + 512],
                        start=True, stop=True,
                    )
                    nc.tensor.matmul(
                        ps2[:, 512:1024], lhsT=w_h,
                        rhs=ys[:, h * 1024 + 512 : (h + 1) * 1024],
                        start=True, stop=True,
                    )
                    nc.scalar.copy(
                        out=ot[:, j * 1024 : j * 1024 + 512],
                        in_=ps2[:, 0:512],
                    )
                    nc.vector.tensor_copy(
                        out=ot[:, j * 1024 + 512 : (j + 1) * 1024],
                        in_=ps2[:, 512:1024],
                    )
                nc.sync.dma_start(
                    out=o_d.ap()[:, hc * CHUNK : (hc + 1) * CHUNK], in_=ot
                )
    _split_excess_waits(nc)
    _NC_CACHE["nc"] = nc
    return nc


def kernel(x, thetas, rotation_matrix, inv_freq, _trace=False):
    from concourse.bass_utils import run_bass_kernel_spmd

    x = np.asarray(x, dtype=np.float32)
    thetas = np.asarray(thetas, dtype=np.float32)
    rotation_matrix = np.asarray(rotation_matrix, dtype=np.float32)
    inv_freq = np.asarray(inv_freq, dtype=np.float32)

    wa, wb = build_weights(thetas, rotation_matrix, inv_freq)
    xs = shard_x(x)

    nc = _build_nc()
    in_maps = [
        {"x": xs[c], "wa": wa[c], "wb": wb} for c in range(NCORES)
    ]
    res = run_bass_kernel_spmd(
        nc, in_maps, core_ids=list(range(NCORES)), trace=_trace
    )
    o = np.stack([res.results[c]["o"] for c in range(NCORES)])
    out = unshard_out(o)
    if _trace:
        return out, res
    return out


# revision 13
# speedup vs baseline: 1.5847x; 1.0128x over previous
"""Trainium2 Bass kernel for nn_CombinedRotaryEmbedding.

v4: the whole op is two cascaded matmuls -- no elementwise stage.

The reference is out = x . M . P(s) per 64-dim head row, where M folds
the 32 Givens rotations + rotation_matrix, and P(s) is the RoPE mixing
(split-even/odd, position-dependent cos/sin).  P(s) = D(s) . Pi with
D(s) block-diagonal 2x2 rotations (angle s*inv_freq[c]) and Pi a fixed
permutation.  Rotations compose: D(a+b) = D(a)D(b).  With s = base(core)
+ 32h + l (l, h in [0,32)):

    out = (x . [M D(base+l)]) . [D(32h) Pi]
           stage 1: 32 weights     stage 2: 32 weights

Each stage is 32 big matmuls (one per l resp. h over 1024 columns), so
the only non-PE work is PSUM->SBUF copies (Act/DVE split).  fp16 I/O
halves HBM traffic (~17 MB/core); total rel err ~4.4e-4 vs the 2e-2
gate.

Sharding: sequence-parallel over 8 cores (1024 positions each).  Host
pre-transposes x to free layout (l, q=(b,hi), h) and inverse-permutes
the output from (h, q, l).
"""

import numpy as np


def _import_bass():
    try:
        import concourse.bass  # noqa: F401
    except ImportError:
        import sys

        sys.path.insert(0, "/opt/trn_rl_repo")


_import_bass()

import concourse.bass as bass  # noqa: E402
import concourse.mybir as mybir  # noqa: E402
from concourse.tile import TileContext  # noqa: E402
from concourse.vector_clock import ScopedClock  # noqa: E402

B, S, NSTATE = 4, 8192, 1024
H, D, NUM_ROT = 16, 64, 32
NCORES = 8
S_SH = S // NCORES  # 1024 positions per core
FREE = B * (H // 2) * S_SH  # 32768 columns per core
CHUNK = 2048

F32 = mybir.dt.float32
F16 = mybir.dt.float16


class _TileContextSplitDrain(TileContext):
    """TileContext whose final drain carries at most one sem wait per
    instruction — the walrus in this container rejects instructions
    with 2+ sync waits ("Too many sync wait commands")."""

    def _drain_and_barrier(self, tick_clock, wait_clock):
        nc = self.nc
        drain_inst = nc.sync.drain()
        wait_clock.add_sem_waits(
            drain_inst.ins, ScopedClock({None: tick_clock.global_clock})
        )
        si = drain_inst.ins.sync_info
        waits = list(si.on_wait or [])
        if len(waits) > 1:
            si.on_wait = [waits[0]]
            for w in waits[1:]:
                n = nc.sync.nop(nofuse=True, hint="drain_wait_split")
                n.ins.sync_info = type(si)(on_update=[], on_wait=[w])
        nc.all_engine_barrier()
        assert self.sems is not None
        popped = nc._tile_sem_poison_stack.pop()
        assert popped is self._sem_poison
        nc.clear_and_free_semaphores(list(self.sems.allocated().values()))
        nc.all_engine_barrier()


def _split_excess_waits(nc, limit=1):
    """Walrus here rejects instructions with >limit sync waits.  Hoist
    excess waits onto same-engine InstNoOps inserted immediately before
    the offending instruction (same engine stream => program order)."""
    n_split = 0
    for fn in nc.m.functions:
        for blk in fn.blocks:
            insts = blk.instructions
            i = 0
            while i < len(insts):
                inst = insts[i]
                si = getattr(inst, "sync_info", None)
                waits = list(si.on_wait) if (si and si.on_wait) else []
                if len(waits) > limit:
                    keep = waits[-limit:]
                    excess = waits[:-limit]
                    si.on_wait = keep
                    for j, w in enumerate(excess):
                        nop = mybir.InstNoOp(
                            name=f"{inst.name}-wsplit{j}",
                            engine=inst.engine,
                            bass_nofuse=True,
                            sync_info=mybir.SyncInfo(on_wait=[w], on_update=[]),
                        )
                        insts.insert(i, nop)
                        i += 1
                        n_split += 1
                i += 1
    return n_split


def compose_rotation(thetas: np.ndarray, rotation_matrix: np.ndarray) -> np.ndarray:
    """Fold the sequential Givens rotations + rotation_matrix into one 64x64."""
    M = np.eye(D, dtype=np.float64)
    th = thetas.astype(np.float64)
    for k in range(NUM_ROT):
        i, j = k % D, (k + 1) % D
        c, s = np.cos(th[k]), np.sin(th[k])
        mi = M[:, i] * c + M[:, j] * s
        mj = -M[:, i] * s + M[:, j] * c
        M[:, i], M[:, j] = mi, mj
    return M @ rotation_matrix.astype(np.float64)


def _dmat(mult: float, invf: np.ndarray) -> np.ndarray:
    """64x64 block-diag over channel pairs (2c,2c+1): [[cos,sin],[-sin,cos]]
    with angle mult*invf[c] (row-vector convention)."""
    Dm = np.zeros((D, D))
    ang = float(mult) * invf.astype(np.float64)
    co, si = np.cos(ang), np.sin(ang)
    for c in range(32):
        Dm[2 * c, 2 * c] = co[c]
        Dm[2 * c, 2 * c + 1] = si[c]
        Dm[2 * c + 1, 2 * c] = -si[c]
        Dm[2 * c + 1, 2 * c + 1] = co[c]
    return Dm


def _pi() -> np.ndarray:
    """RoPE output permutation: channel 2c -> c, 2c+1 -> 32+c."""
    P = np.zeros((D, D))
    for c in range(32):
        P[2 * c, c] = 1.0
        P[2 * c + 1, 32 + c] = 1.0
    return P


def _pack2(W: np.ndarray) -> np.ndarray:
    """[64,64] -> [128,128] block-diag x2 (two head-halves share weights)."""
    Z = np.zeros((128, 128))
    Z[0:64, 0:64] = W
    Z[64:128, 64:128] = W
    return Z


def build_weights(thetas, rotation_matrix, inv_freq):
    """Per-core stage-1 stacks [8,128,4096] and shared stage-2 [128,4096]."""
    M = compose_rotation(thetas, rotation_matrix)
    invf = inv_freq.astype(np.float64)
    wa = np.empty((NCORES, 128, 32 * 128), dtype=np.float16)
    for c in range(NCORES):
        base = S_SH * c
        for l in range(32):
            wa[c, :, l * 128 : (l + 1) * 128] = _pack2(M @ _dmat(base + l, invf))
    Pi = _pi()
    wb = np.empty((128, 32 * 128), dtype=np.float16)
    for h in range(32):
        wb[:, h * 128 : (h + 1) * 128] = _pack2(_dmat(32 * h, invf) @ Pi)
    return wa, wb


def shard_x(x: np.ndarray) -> np.ndarray:
    """[B,S,1024] f32 -> [core, 128 (hp,d), 32768 (l, h, q=(b,hi))] fp16."""
    xr = np.ascontiguousarray(x).reshape(B, NCORES, 32, 32, H // 2, 2, D)
    # dims: b, core, h, l, hi, hp, d  ->  core, hp, d, l, h, b, hi
    xt = xr.transpose(1, 5, 6, 3, 2, 0, 4)
    return np.ascontiguousarray(xt, dtype=np.float16).reshape(NCORES, 128, FREE)


def unshard_out(o: np.ndarray) -> np.ndarray:
    """[core, 128 (hp,c), 32768 (h, l, q=(b,hi))] fp16 -> [B,S,1024] f32."""
    orr = o.astype(np.float32).reshape(NCORES, 2, 64, 32, 32, B, 8)
    # dims: core, hp, c, h, l, b, hi -> b, (core,h,l)=s, (hi,hp,c)=n
    ot = orr.transpose(5, 0, 3, 4, 6, 1, 2)
    return np.ascontiguousarray(ot).reshape(B, S, NSTATE)


_NC_CACHE = {}


def _build_nc():
    if "nc" in _NC_CACHE:
        return _NC_CACHE["nc"]
    nc = bass.Bass(trn_type="TRN2")
    x_d = nc.dram_tensor("x", [128, FREE], F16, kind="ExternalInput")
    wa_d = nc.dram_tensor("wa", [128, 4096], F16, kind="ExternalInput")
    wb_d = nc.dram_tensor("wb", [128, 4096], F16, kind="ExternalInput")
    o_d = nc.dram_tensor("o", [128, FREE], F16, kind="ExternalOutput")

    with _TileContextSplitDrain(nc) as tc:
        with tc.tile_pool(name="const", bufs=1) as cpool, \
             tc.tile_pool(name="xin", bufs=6) as xpool, \
             tc.tile_pool(name="ys", bufs=1) as ypool, \
             tc.tile_pool(name="oout", bufs=4) as opool, \
             tc.tile_pool(name="ps1", bufs=2, space="PSUM") as p1pool, \
             tc.tile_pool(name="ps2", bufs=2, space="PSUM") as p2pool:
            wa = cpool.tile([128, 4096], F16, tag="wa")
            wb = cpool.tile([128, 4096], F16, tag="wb")
            nc.sync.dma_start(out=wa[:, 0:2048], in_=wa_d.ap()[:, 0:2048])

            # ys free layout m*1024 + q*32 + l (m-major) so stage-2 rhs is
            # CONTIGUOUS -- strided matmul rhs measured +70% PE time.  The
            # scatter moves into the stage-1 drain instead (copies tolerate
            # strides; the PE does not).
            ys = ypool.tile([128, FREE], F16, tag="ys")
            ysv = ys.rearrange("p (m l q) -> p m l q", m=32, l=32, q=32)

            # Stage 1: per l-block, u = x . M D(base+l); scatter into ys.
            for ch in range(FREE // CHUNK):
                xt = xpool.tile([128, CHUNK], F16)
                nc.sync.dma_start(
                    out=xt, in_=x_d.ap()[:, ch * CHUNK : (ch + 1) * CHUNK]
                )
                if ch == 0:
                    # Later-needed weights load behind the first x chunk
                    # instead of delaying stage 1.
                    nc.sync.dma_start(
                        out=wa[:, 2048:4096], in_=wa_d.ap()[:, 2048:4096]
                    )
                    nc.sync.dma_start(out=wb, in_=wb_d.ap())
                for j in range(CHUNK // 1024):
                    l = ch * (CHUNK // 1024) + j
                    w_l = wa[:, l * 128 : (l + 1) * 128]
                    ps1 = p1pool.tile([128, 1024], F32)
                    nc.tensor.matmul(
                        ps1[:, 0:512], lhsT=w_l,
                        rhs=xt[:, j * 1024 : j * 1024 + 512],
                        start=True, stop=True,
                    )
                    nc.tensor.matmul(
                        ps1[:, 512:1024], lhsT=w_l,
                        rhs=xt[:, j * 1024 + 512 : (j + 1) * 1024],
                        start=True, stop=True,
                    )
                    p1v = ps1.rearrange("p (m q) -> p m q", m=32, q=32)
                    nc.scalar.copy(
                        out=ysv[:, 0:16, l, :], in_=p1v[:, 0:16, :]
                    )
                    nc.vector.tensor_copy(
                        out=ysv[:, 16:32, l, :], in_=p1v[:, 16:32, :]
                    )

            # Stage 2: per h-block, out = ys_h . D(32h) Pi; contiguous rhs
            # and contiguous drain into ot (free layout h*1024 + q*32 + l).
            for hc in range(FREE // CHUNK):
                ot = opool.tile([128, CHUNK], F16)
                for j in range(CHUNK // 1024):
                    h = hc * (CHUNK // 1024) + j
                    w_h = wb[:, h * 128 : (h + 1) * 128]
                    ps2 = p2pool.tile([128, 1024], F32)
                    nc.tensor.matmul(
                        ps2[:, 0:512], lhsT=w_h,
                        rhs=ys[:, h * 1024 : h * 1024 + 512],
                        start=True, stop=True,
                    )
                    nc.tensor.matmul(
                        ps2[:, 512:1024], lhsT=w_h,
                        rhs=ys[:, h * 1024 + 512 : (h + 1) * 1024],
                        start=True, stop=True,
                    )
                    nc.scalar.copy(
                        out=ot[:, j * 1024 : j * 1024 + 512],
                        in_=ps2[:, 0:512],
                    )
                    nc.vector.tensor_copy(
                        out=ot[:, j * 1024 + 512 : (j + 1) * 1024],
                        in_=ps2[:, 512:1024],
                    )
                nc.sync.dma_start(
                    out=o_d.ap()[:, hc * CHUNK : (hc + 1) * CHUNK], in_=ot
                )
    _split_excess_waits(nc)
    _NC_CACHE["nc"] = nc
    return nc


def kernel(x, thetas, rotation_matrix, inv_freq, _trace=False):
    from concourse.bass_utils import run_bass_kernel_spmd

    x = np.asarray(x, dtype=np.float32)
    thetas = np.asarray(thetas, dtype=np.float32)
    rotation_matrix = np.asarray(rotation_matrix, dtype=np.float32)
    inv_freq = np.asarray(inv_freq, dtype=np.float32)

    wa, wb = build_weights(thetas, rotation_matrix, inv_freq)
    xs = shard_x(x)

    nc = _build_nc()
    in_maps = [
        {"x": xs[c], "wa": wa[c], "wb": wb} for c in range(NCORES)
    ]
    res = run_bass_kernel_spmd(
        nc, in_maps, core_ids=list(range(NCORES)), trace=_trace
    )
    o = np.stack([res.results[c]["o"] for c in range(NCORES)])
    out = unshard_out(o)
    if _trace:
        return out, res
    return out
